# revision 22
# baseline (speedup 1.0000x reference)
"""Trainium2 Bass kernel for nn_Attention_34351148434119 (8 NeuronCores).

Reference computation (faithful quirks included):
  q_proj = hid @ Wq; q, gate = split(q_proj)     # q is DEAD code downstream
  k = hid @ Wk; v = hid @ Wv                     # [B,KV,S,D]
  v = RoPE(v)  (k is NOT roped; q roped but unused)
  scores = (k @ v^T) * sqrt(D) + mask; attn = softmax_t(scores)   # per kv head
  out = (tile_G(attn @ v) * sigmoid(gate)) @ Wo

Sharding: core = b*4 + j  (b = batch, j = rank in 4-core batch group).
Per batch, S=2048 is split into 16 blocks of 128 rows; core j owns blocks
{4k+j} (slot k) so every core has an identical causal workload (uniform
SPMD graph).

Phase order is chosen to hide the v AllGather (measured ~60-100us wall
on the CC stream): v projection runs FIRST, the RoPE'd v is packed
hi/lo-bf16 d-major + bf16 row-major into ONE fused AllGather, and the
k projection + gate matmul (whose weights are resident / independently
streamed) execute while the collective flies.

Precision: logits have sigma~105 (SCALING MULTIPLIES by sqrt(D)), so
bf16 anywhere in the k/v->scores chain flips softmax argmax rows.  All
GEMMs run bf16 with hi/lo splitting where >8 mantissa bits are needed:
k/v projections use a 3-pass hi/lo scheme (~16-bit effective), and
scores = khi@vhi + klo@vhi + khi@vlo (3 bf16 passes, 25% cheaper than
one fp32 pass).  hi/lo of hidden/Wk/Wv are precomputed on the host.
Softmax is two-phase (chunk maxima, one exp wave with the final max as
ACT bias); attn stays UNSCALED through the PE transpose so transposes
don't wait on the row sums, and 1/sum enters once per kv head via a
transposed per-slot broadcast multiplied into attn@v's PSUM result.
"""
import sys
import numpy as np

sys.path.insert(0, "/opt/trn_rl_repo")

B, S, HS = 2, 2048, 2048
H, KV, D = 16, 4, 128
G = H // KV
SCALING = float(D) ** 0.5
P = 128
NB = S // P            # 16 row blocks per batch
NCORES = 8
RANKS = 4              # cores per batch group
SLOTS = 4              # owned 128-row blocks per core
ROWS = SLOTS * P       # 512 rows per core
CHUNK = 512            # t-chunk = 4 t-tiles
NCHUNK = S // CHUNK    # 4
KT = HS // P           # 16 contraction tiles
NEG_THRESH = -1e8

_CACHE = {}


def _mask_classes(mask):
    """Classify each (s-slot k, t-chunk c) 512x512 region of the SxS mask.

    0 = skip (everything <= NEG_THRESH: contributes exact 0 after softmax)
    1 = plain (all zeros: no add needed)
    2 = add  (mixed: stage values and add on-chip)
    Slot k rows across all cores = blocks 4k..4k+3 = rows [512k, 512k+512).
    """
    cls = [[0] * NCHUNK for _ in range(SLOTS)]
    for k in range(SLOTS):
        for c in range(NCHUNK):
            reg = mask[512 * k:512 * (k + 1), 512 * c:512 * (c + 1)]
            if (reg <= NEG_THRESH).all():
                cls[k][c] = 0
            elif (reg == 0).all():
                cls[k][c] = 1
            else:
                cls[k][c] = 2
    ok = True
    for k in range(SLOTS):
        comp = [c for c in range(NCHUNK) if cls[k][c] != 0]
        # computed chunks must be a prefix starting at 0
        if comp != list(range(len(comp))) or 0 not in comp:
            ok = False
    if ok:
        # {k : chunk c computed} must be a suffix of slots for each c
        for c in range(NCHUNK):
            ks = [k for k in range(SLOTS) if cls[k][c] != 0]
            if ks != list(range(SLOTS - len(ks), SLOTS)):
                ok = False
    if not ok:
        # fully dense fallback: always correct for any mask
        cls = [[2] * NCHUNK for _ in range(SLOTS)]
    return cls


def _build(classes):
    from contextlib import ExitStack

    from concourse import bacc, mybir, tile
    from concourse.masks import make_identity

    f32 = mybir.dt.float32
    bf16 = mybir.dt.bfloat16
    Alu = mybir.AluOpType
    Act = mybir.ActivationFunctionType

    computed = [[c for c in range(NCHUNK) if classes[k][c] != 0] for k in range(SLOTS)]
    add_idx = {}
    for k in range(SLOTS):
        for c in range(NCHUNK):
            if classes[k][c] == 2:
                add_idx[(k, c)] = len(add_idx)
    n_add = max(len(add_idx), 1)
    dense = len(add_idx) > 6

    # attn@v wave plan (same for every g): process slots high->low; emit
    # block bi's matmuls as soon as every slot column it reads is
    # transposed.  Per 128-col psum block kb, track first/last writer so
    # PSUM accumulation start/stop flags are exact.
    slot_order = sorted(range(SLOTS), key=lambda k: -len(computed[k]))
    ks_of = {}
    for bi in range(NB):
        c = bi // RANKS
        ks = [k for k in range(SLOTS) if c in computed[k]]
        if ks:
            ks_of[bi] = ks
    emit_at = {k: [] for k in slot_order}   # slot step -> [bi]
    done = set()
    seen = set()
    for k in slot_order:
        seen.add(k)
        for bi in range(NB):
            if bi in done or bi not in ks_of:
                continue
            if set(ks_of[bi]) <= seen:
                emit_at[k].append(bi)
                done.add(bi)
    # per-block first/last writer (bi, kb) in emission order
    writers = {}
    for k in slot_order:
        for bi in emit_at[k]:
            for kb in ks_of[bi]:
                writers.setdefault(kb, []).append(bi)

    nc = bacc.Bacc("TRN2", target_bir_lowering=False, debug=False,
                   num_devices=NCORES)

    hidhi_d = nc.declare_dram_parameter("hidhi", [HS, ROWS], bf16, isOutput=False)
    hidlo_d = nc.declare_dram_parameter("hidlo", [HS, ROWS], bf16, isOutput=False)
    # per-projection packed weights: cols [0:512] hi, [512:1024] lo
    wvv_d = nc.declare_dram_parameter("wvv", [HS, 2 * KV * D], bf16, isOutput=False)
    wkk_d = nc.declare_dram_parameter("wkk", [HS, 2 * KV * D], bf16, isOutput=False)
    wqg_d = nc.declare_dram_parameter("wqg", [HS, HS], bf16, isOutput=False)
    wo_d = nc.declare_dram_parameter("wo", [HS, HS], bf16, isOutput=False)
    cosT_d = nc.declare_dram_parameter("cosT", [D, ROWS], f32, isOutput=False)
    sinT_d = nc.declare_dram_parameter("sinT", [D, ROWS], f32, isOutput=False)
    mask_d = nc.declare_dram_parameter("maskst", [n_add, P, CHUNK], bf16,
                                       isOutput=False)
    out_d = nc.declare_dram_parameter("out", [ROWS, HS], bf16, isOutput=True)

    rg = [[0, 1, 2, 3], [4, 5, 6, 7]]

    with tile.TileContext(nc) as tc, ExitStack() as ctx:
        sb = ctx.enter_context(tc.tile_pool(name="sb", bufs=2))
        ps = ctx.enter_context(tc.tile_pool(name="ps", bufs=8, space="PSUM"))
        dram = ctx.enter_context(tc.tile_pool(name="dram", bufs=1, space="DRAM"))

        # ---- constants ----
        id_bf = sb.tile([P, P], bf16, tag="c_idb")
        make_identity(nc, id_bf[:])
        ones_bf = sb.tile([P, P], bf16, tag="c_ones")
        nc.vector.memset(ones_bf[:], 1.0)
        cosT = sb.tile([D, ROWS], f32, tag="c_cos")
        sinT = sb.tile([D, ROWS], f32, tag="c_sin")

        # ---- v projection (3-pass hi/lo bf16; wv + hid-lo streamed) ----
        hidhi = []
        pv = [ps.tile([P, ROWS], f32, tag="ps", name=f"pv{g}") for g in range(KV)]
        for kk in range(KT):
            if kk == 4:
                nc.sync.dma_start(cosT[:], cosT_d[:, :])
                nc.sync.dma_start(sinT[:], sinT_d[:, :])
            hh = sb.tile([P, ROWS], bf16, tag="hidhi", bufs=KT)
            nc.sync.dma_start(hh[:], hidhi_d[kk * P:(kk + 1) * P, :])
            hidhi.append(hh)
            hl = sb.tile([P, ROWS], bf16, tag="hidlo", bufs=6, name=f"hlv{kk}")
            nc.sync.dma_start(hl[:], hidlo_d[kk * P:(kk + 1) * P, :])
            wt = sb.tile([P, 2 * KV * D], bf16, tag="wvv", bufs=4, name=f"wv{kk}")
            nc.sync.dma_start(wt[:], wvv_d[kk * P:(kk + 1) * P, :])
            for g in range(KV):
                whi = wt[:, g * P:(g + 1) * P]
                wlo = wt[:, KV * D + g * P:KV * D + (g + 1) * P]
                nc.tensor.matmul(pv[g][:], whi, hh[:], start=(kk == 0), stop=False)
                nc.tensor.matmul(pv[g][:], wlo, hh[:], start=False, stop=False)
                nc.tensor.matmul(pv[g][:], whi, hl[:], start=False,
                                 stop=(kk == KT - 1))

        # ---- RoPE v (f32), split hi/lo bf16, AG pack ----
        agi = dram.tile([ROWS, 2 * CHUNK], bf16)
        ago = dram.tile([RANKS * ROWS, 2 * CHUNK], bf16)
        for g in range(KV):
            vr = sb.tile([P, ROWS], f32, tag="vraw", bufs=2)
            nc.scalar.copy(vr[:], pv[g][:])
            rot = sb.tile([P, ROWS], f32, tag="vrot", bufs=2)
            nc.vector.tensor_scalar_mul(rot[0:64, :], vr[64:128, :], -1.0)
            nc.gpsimd.tensor_copy(rot[64:128, :], vr[0:64, :])
            nc.vector.tensor_mul(vr[:], vr[:], cosT[:])
            nc.gpsimd.tensor_mul(rot[:], rot[:], sinT[:])
            nc.vector.tensor_add(vr[:], vr[:], rot[:])
            hi = sb.tile([P, ROWS], bf16, tag="vhi", bufs=2)
            nc.scalar.copy(hi[:], vr[:])
            lo = sb.tile([P, ROWS], bf16, tag="vlo", bufs=2)
            nc.vector.tensor_sub(lo[:], vr[:], hi[:])
            nc.sync.dma_start(agi[g * P:(g + 1) * P, 0:CHUNK], hi[:])
            nc.sync.dma_start(agi[g * P:(g + 1) * P, CHUNK:2 * CHUNK], lo[:])
        # mask strips first on gpsimd so they don't queue behind the AG
        mts = []
        for i in range(n_add):
            mt = sb.tile([P, CHUNK], bf16, tag="msk", bufs=n_add, name=f"msk{i}")
            nc.gpsimd.dma_start(mt[:], mask_d[i, :, :])
            mts.append(mt)
        nc.gpsimd.collective_compute(
            "AllGather", mybir.AluOpType.bypass, replica_groups=rg,
            ins=[agi.opt()], outs=[ago.opt()])

        # ---- gathered-v retrieval (gpsimd: the only AG-dependent stream) ----
        # vthl[g]: [128 d, (hl, chunk c, rank r, 128)] bf16; one DMA per (g, r).
        vthl = [sb.tile([P, 2, NCHUNK, RANKS, P], bf16, tag="vthl", bufs=KV,
                        name=f"vthl{g}") for g in range(KV)]
        for g in range(KV):
            for r in range(RANKS):
                src = ago[ROWS * r + g * P:ROWS * r + (g + 1) * P, 0:2 * CHUNK]
                src = src.rearrange("p (hl c i) -> p hl c i", hl=2, c=NCHUNK, i=P)
                nc.gpsimd.dma_start(vthl[g][:, :, :, r, :], src)
        # vrg[bi]: [128 t, 512 d(g-major)] bf16 row-major v, derived locally
        # by PE transposes of the gathered d-major hi (column g filled inside
        # g's attention section, just before its waves need it)
        vrg = [sb.tile([P, KV * D], bf16, tag="vrg", bufs=NB, name=f"vrg{i}")
               for i in range(NB)]

        # ---- k projection (runs while the AG flies) ----
        pk = [ps.tile([P, ROWS], f32, tag="ps", name=f"pk{g}") for g in range(KV)]
        for kk in range(KT):
            wt = sb.tile([P, 2 * KV * D], bf16, tag="wvv", bufs=4, name=f"wk{kk}")
            nc.sync.dma_start(wt[:], wkk_d[kk * P:(kk + 1) * P, :])
            hl = sb.tile([P, ROWS], bf16, tag="hidlo", bufs=6, name=f"hlk{kk}")
            nc.sync.dma_start(hl[:], hidlo_d[kk * P:(kk + 1) * P, :])
            for g in range(KV):
                whi = wt[:, g * P:(g + 1) * P]
                wlo = wt[:, KV * D + g * P:KV * D + (g + 1) * P]
                nc.tensor.matmul(pk[g][:], whi, hidhi[kk][:], start=(kk == 0),
                                 stop=False)
                nc.tensor.matmul(pk[g][:], wlo, hidhi[kk][:], start=False,
                                 stop=False)
                nc.tensor.matmul(pk[g][:], whi, hl[:], start=False,
                                 stop=(kk == KT - 1))
        khi = []
        klo = []
        for g in range(KV):
            hi = sb.tile([P, ROWS], bf16, tag="khi", bufs=KV)
            nc.scalar.mul(hi[:], pk[g][:], SCALING)
            lo = sb.tile([P, ROWS], bf16, tag="klo", bufs=KV)
            nc.vector.scalar_tensor_tensor(lo[:], pk[g][:], SCALING, hi[:],
                                           Alu.mult, Alu.subtract)
            khi.append(hi)
            klo.append(lo)

        # ---- gate matmul (per-nblk bf16 slabs streamed post-AG-bulk) ----
        sigT = [None] * H
        for nblk in range(4):
            wqb = []
            for kk in range(KT):
                wt = sb.tile([P, CHUNK], bf16, tag="wq", bufs=16,
                             name=f"wq{nblk}_{kk}")
                nc.sync.dma_start(
                    wt[:], wqg_d[kk * P:(kk + 1) * P,
                                 nblk * CHUNK:(nblk + 1) * CHUNK])
                wqb.append(wt)
            for m in range(4):
                pg = ps.tile([P, ROWS], f32, tag="ps", name=f"pg{nblk}_{m}")
                for kk in range(KT):
                    nc.tensor.matmul(pg[:], wqb[kk][:, m * P:(m + 1) * P],
                                     hidhi[kk][:], start=(kk == 0),
                                     stop=(kk == KT - 1))
                t = sb.tile([P, ROWS], bf16, tag="sg", bufs=20)
                nc.scalar.activation(t[:], pg[:], Act.Sigmoid)
                sigT[nblk * 4 + m] = t

        # ---- attention per kv head ----
        gat = [None] * H
        for g in range(KV):
            # fill vrg column g: transpose gathered d-major hi per t-block
            for c in range(NCHUNK):
                for r in range(RANKS):
                    tpv = ps.tile([P, P], bf16, tag="ps")
                    nc.tensor.transpose(tpv[:], vthl[g][:, 0, c, r, :], id_bf[:])
                    if (c + r) % 2:
                        nc.scalar.copy(
                            vrg[RANKS * c + r][:, g * P:(g + 1) * P], tpv[:])
                    else:
                        nc.vector.tensor_copy(
                            vrg[RANKS * c + r][:, g * P:(g + 1) * P], tpv[:])
            attnT = [sb.tile([P, ROWS], bf16, tag="attnT", bufs=NB,
                             name=f"attnT{g}_{bi}") for bi in range(NB)]
            rb = sb.tile([P, ROWS], bf16, tag="rb", bufs=2, name=f"rb{g}")
            # attn@v accumulates in SBUF (one short-lived PSUM tile per wave:
            # accumulation groups must not interleave within a PSUM bank)
            avacc = sb.tile([P, ROWS], f32, tag="avacc", bufs=2, name=f"avacc{g}")
            fresh = set(writers)   # blocks not yet written to avacc
            writ = {kb: list(v) for kb, v in writers.items()}
            for k in slot_order:
                comp = computed[k]
                nchk = len(comp)
                attn = sb.tile([P, CHUNK * nchk], bf16, tag="attn",
                               bufs=3,
                               padded_shape=[P, CHUNK * NCHUNK],
                               name=f"attn{g}_{k}")
                pscs = []
                cms = []
                for ci, c in enumerate(comp):
                    psc = ps.tile([P, CHUNK], f32, tag="ps", name=f"psc{ci}")
                    vh = vthl[g][:, 0, c]
                    vl = vthl[g][:, 1, c]
                    sh = khi[g][:, k * P:(k + 1) * P]
                    sl = klo[g][:, k * P:(k + 1) * P]
                    nc.tensor.matmul(psc[:], sh, vh, start=True, stop=False)
                    nc.tensor.matmul(psc[:], sl, vh, start=False, stop=False)
                    nc.tensor.matmul(psc[:], sh, vl, start=False, stop=True)
                    if classes[k][c] == 2:
                        nc.vector.tensor_add(psc[:], psc[:],
                                             mts[add_idx[(k, c)]][:])
                    cm = sb.tile([P, 1], f32, tag="stat", bufs=32, name=f"cm{ci}")
                    nc.vector.tensor_reduce(cm[:], psc[:], mybir.AxisListType.X,
                                            Alu.max, negate=True)
                    pscs.append(psc)
                    cms.append(cm)
                mneg = cms[0]   # -max
                for ci in range(1, nchk):
                    mnew = sb.tile([P, 1], f32, tag="stat", bufs=32, name=f"mn{ci}")
                    nc.vector.tensor_tensor(mnew[:], mneg[:], cms[ci][:], Alu.min)
                    mneg = mnew
                tot = None
                for ci in range(nchk):
                    csum = sb.tile([P, 1], f32, tag="stat", bufs=32, name=f"cs{ci}")
                    nc.scalar.activation(attn[:, ci * CHUNK:(ci + 1) * CHUNK],
                                         pscs[ci][:], Act.Exp, bias=mneg[:],
                                         accum_out=csum[:])
                    if tot is None:
                        tot = csum
                    else:
                        t2 = sb.tile([P, 1], f32, tag="stat", bufs=32,
                                     name=f"tt{ci}")
                        nc.vector.tensor_add(t2[:], tot[:], csum[:])
                        tot = t2
                # transpose UNSCALED attn -> attnT column slot k (no sum dep)
                for ci, c in enumerate(comp):
                    for i in range(4):
                        bi = 4 * c + i
                        tp = ps.tile([P, P], bf16, tag="ps")
                        nc.tensor.transpose(
                            tp[:],
                            attn[:, ci * CHUNK + i * P:ci * CHUNK + (i + 1) * P],
                            id_bf[:])
                        if i % 2:
                            nc.scalar.copy(attnT[bi][:, k * P:(k + 1) * P], tp[:])
                        else:
                            nc.vector.tensor_copy(attnT[bi][:, k * P:(k + 1) * P],
                                                  tp[:])
                # 1/sum -> transposed broadcast column of rb
                rinv = sb.tile([P, 1], f32, tag="stat", bufs=32)
                nc.vector.reciprocal(rinv[:], tot[:])
                bc = sb.tile([P, P], bf16, tag="bc", bufs=4)
                nc.scalar.activation(bc[:], ones_bf[:], Act.Copy, scale=rinv[:])
                tpb = ps.tile([P, P], bf16, tag="ps")
                nc.tensor.transpose(tpb[:], bc[:], id_bf[:])
                nc.vector.tensor_copy(rb[:, k * P:(k + 1) * P], tpb[:])
                # attn@v wave: blocks whose every slot column is now ready.
                # One PSUM tile per wave; per-kb groups SEQUENTIAL in the bank.
                wbis = emit_at[k]
                if wbis:
                    kbs = sorted({kb for bi in wbis for kb in ks_of[bi]})
                    wps = ps.tile([P, len(kbs) * P], f32, tag="ps",
                                  name=f"wps{g}_{k}")
                    for wi, kb in enumerate(kbs):
                        bis = [bi for bi in wbis if kb in ks_of[bi]]
                        for bi in bis:
                            writ[kb].remove(bi)
                            nc.tensor.matmul(
                                wps[:, wi * P:(wi + 1) * P],
                                vrg[bi][:, g * P:(g + 1) * P],
                                attnT[bi][:, kb * P:(kb + 1) * P],
                                start=(bi == bis[0]), stop=(bi == bis[-1]))
                    # fold the wave into the SBUF accumulator (runs of
                    # contiguous kb with the same fresh/accumulate kind)
                    merged = []
                    for wi, kb in enumerate(kbs):
                        kind = kb in fresh
                        fresh.discard(kb)
                        if merged and merged[-1][2] == kind and \
                                kb == merged[-1][0] + merged[-1][3] and \
                                wi == merged[-1][1] + merged[-1][3]:
                            merged[-1][3] += 1
                        else:
                            merged.append([kb, wi, kind, 1])
                    for kb, wi, kind, n in merged:
                        dst = avacc[:, kb * P:(kb + n) * P]
                        src = wps[:, wi * P:(wi + n) * P]
                        if kind:
                            nc.vector.tensor_copy(dst, src)
                        else:
                            nc.vector.tensor_add(dst, dst, src)
            avt = sb.tile([P, ROWS], bf16, tag="avT", bufs=2)
            nc.vector.tensor_tensor(avt[:], avacc[:], rb[:], Alu.mult)
            for i in range(G):
                t = sb.tile([P, ROWS], bf16, tag="sg", bufs=20)
                nc.vector.tensor_mul(t[:], avt[:], sigT[4 * g + i][:])
                gat[4 * g + i] = t

        # ---- out projection (bf16; wo slabs streamed on the gpsimd queue) ----
        for nblk in range(4):
            wob = []
            for cc in range(KT):
                t = sb.tile([P, CHUNK], bf16, tag="wo", bufs=14,
                            name=f"wo{nblk}_{cc}")
                nc.gpsimd.dma_start(
                    t[:], wo_d[cc * P:(cc + 1) * P,
                               nblk * CHUNK:(nblk + 1) * CHUNK])
                wob.append(t)
            for rt in range(SLOTS):
                po = ps.tile([P, CHUNK], f32, tag="ps")
                for cc in range(KT):
                    nc.tensor.matmul(po[:], gat[cc][:, rt * P:(rt + 1) * P],
                                     wob[cc][:], start=(cc == 0),
                                     stop=(cc == KT - 1))
                t = sb.tile([P, CHUNK], bf16, tag="oev", bufs=2)
                if rt % 2:
                    nc.vector.tensor_copy(t[:], po[:])
                else:
                    nc.scalar.copy(t[:], po[:])
                nc.sync.dma_start(
                    out_d[rt * P:(rt + 1) * P, nblk * CHUNK:(nblk + 1) * CHUNK],
                    t[:])

    nc.compile()
    return nc


def kernel(hidden_states, cos, sin, attention_mask, Wq, Wk, Wv, Wo):
    import ml_dtypes
    from concourse.bass_utils import run_bass_kernel_spmd

    bf = ml_dtypes.bfloat16
    hidden_states = np.asarray(hidden_states, dtype=np.float32)
    cos = np.asarray(cos, dtype=np.float32)
    sin = np.asarray(sin, dtype=np.float32)
    mask = np.asarray(attention_mask, dtype=np.float32)[0, 0]
    Wq = np.asarray(Wq, dtype=np.float32)
    Wk = np.asarray(Wk, dtype=np.float32)
    Wv = np.asarray(Wv, dtype=np.float32)
    Wo = np.asarray(Wo, dtype=np.float32)

    classes = _mask_classes(mask)
    key = tuple(tuple(r) for r in classes)
    if key not in _CACHE:
        _CACHE[key] = _build(classes)
    nc = _CACHE[key]

    def hilo(x):
        hi = x.astype(bf)
        lo = (x - hi.astype(np.float32)).astype(bf)
        return hi, lo

    wv_hi, wv_lo = hilo(Wv)
    wk_hi, wk_lo = hilo(Wk)
    wvv_p = np.ascontiguousarray(np.concatenate([wv_hi, wv_lo], axis=1))
    wkk_p = np.ascontiguousarray(np.concatenate([wk_hi, wk_lo], axis=1))
    wqg = np.ascontiguousarray(Wq[:, HS:].astype(bf))
    wo16 = np.ascontiguousarray(Wo.astype(bf))

    in_maps = []
    for core in range(NCORES):
        b, j = divmod(core, RANKS)
        blocks = [RANKS * k + j for k in range(SLOTS)]
        rows = np.concatenate([np.arange(bi * P, (bi + 1) * P) for bi in blocks])
        hidT = np.ascontiguousarray(hidden_states[b][rows].T)
        hid_hi, hid_lo = hilo(hidT)
        strips = []
        for k in range(SLOTS):
            for c in range(NCHUNK):
                if classes[k][c] == 2:
                    bi = RANKS * k + j
                    strips.append(mask[bi * P:(bi + 1) * P,
                                       c * CHUNK:(c + 1) * CHUNK])
        if not strips:
            strips.append(np.zeros((P, CHUNK), np.float32))
        in_maps.append({
            "hidhi": np.ascontiguousarray(hid_hi),
            "hidlo": np.ascontiguousarray(hid_lo),
            "wvv": wvv_p,
            "wkk": wkk_p,
            "wqg": wqg,
            "wo": wo16,
            "cosT": np.ascontiguousarray(cos[b][rows].T),
            "sinT": np.ascontiguousarray(sin[b][rows].T),
            "maskst": np.ascontiguousarray(np.stack(strips).astype(bf)),
        })

    res = run_bass_kernel_spmd(nc, in_maps, core_ids=list(range(NCORES)))

    out = np.empty((B, S, HS), np.float32)
    for core in range(NCORES):
        b, j = divmod(core, RANKS)
        o = np.asarray(res.results[core]["out"]).astype(np.float32)
        for k in range(SLOTS):
            bi = RANKS * k + j
            out[b, bi * P:(bi + 1) * P, :] = o[k * P:(k + 1) * P, :]
    return out
